# revision 1
# baseline (speedup 1.0000x reference)
"""Optimized Trainium2 Bass kernel for nn_GCN_11269994185058 (v2).

Changes vs v1 baseline:
- bf16 table (rows padded to 128 cols = 256B gather granularity), bf16
  gathered tiles, bf16 one-hots -> PE fast-weight-load + DVE 4x mode.
- Symmetric-norm dinv[src]*dinv[dst] folded into the one-hot VALUES via the
  two-scalar tensor_scalar (op0=is_equal, op1=mult) -> z' scale, dinv mult,
  and per-block adds all disappear.
- Both table halves accumulate into one PSUM tile per dest block; tanh+bias
  applied by ScalarE directly from PSUM.
- Configurable gather call size (CALL_TILES tiles of 128 idxs per call).
"""

from contextlib import ExitStack

import numpy as np

import concourse.bass as bass
import concourse.mybir as mybir
from concourse import tile
from concourse.library_config import mlp
from concourse.library_overlay import lower_extended_insts
from concourse.tile import add_dep_helper

NCORES = 8
D = 64
DP = 128             # padded table row width (bf16) -> 256B
PART = 128
TILE_E = 128
NEG_BIG = -1.0e30


# ======================= host-side preprocessing =======================


def preprocess(x, edge_index, batch, call_tiles=8):
    N, D_IN = x.shape
    G = int(batch.max()) + 1
    gpc = G // NCORES
    assert gpc == PART

    counts = np.bincount(batch, minlength=G)
    S = int(counts.max())
    npc = PART * S
    NB = npc // PART
    NT = NCORES * npc
    half = NT // 2
    assert half <= 32768

    gstart = np.zeros(G + 1, np.int64)
    np.cumsum(counts, out=gstart[1:])
    g_of_n = batch.astype(np.int64)
    j_of_n = np.arange(N) - gstart[g_of_n]
    core_of_n = g_of_n // gpc
    loc_of_n = (g_of_n % gpc) * S + j_of_n
    pgid_of_n = core_of_n * npc + loc_of_n

    deg = np.bincount(edge_index[1], minlength=N).astype(np.float64) + 1.0
    dinv_n = (1.0 / np.sqrt(deg)).astype(np.float32)

    src = np.concatenate([edge_index[0], np.arange(N, dtype=edge_index.dtype)])
    dst = np.concatenate([edge_index[1], np.arange(N, dtype=edge_index.dtype)])
    e_norm = (dinv_n[src] * dinv_n[dst]).astype(np.float32)
    e_src_pgid = pgid_of_n[src]
    e_dst_core = core_of_n[dst]
    e_dst_loc = loc_of_n[dst]
    e_half = e_src_pgid // half
    e_block = e_dst_loc // TILE_E

    cnt = np.zeros((NCORES, 2, NB), np.int64)
    np.add.at(cnt, (e_dst_core, e_half, e_block), 1)
    tiles_hb = np.maximum(1, -(-cnt.max(axis=0) // TILE_E))    # [2, NB]

    tiles_per_half = tiles_hb.sum(axis=1)
    ncalls_h = [int(-(-t // call_tiles)) for t in tiles_per_half]
    TT = int(sum(ncalls_h)) * call_tiles

    per_core = []
    for k in range(NCORES):
        sel = e_dst_core == k
        ksrc = e_src_pgid[sel]
        kloc = e_dst_loc[sel]
        khalf = e_half[sel]
        kblk = e_block[sel]
        knorm = e_norm[sel]
        ksrc_half = ksrc - khalf * half
        order = np.lexsort((ksrc, kblk, khalf))
        ksrc_half = ksrc_half[order]
        kloc = kloc[order]
        khalf = khalf[order]
        kblk = kblk[order]
        knorm = knorm[order]

        gidx_cols = []
        doff_cols = []
        norm_cols = []
        for h in (0, 1):
            idx_h, doff_h, norm_h = [], [], []
            for b in range(NB):
                m = (khalf == h) & (kblk == b)
                ns = int(m.sum())
                cap = int(tiles_hb[h, b]) * TILE_E
                assert ns <= cap
                iv = np.zeros(cap, np.int16)
                dv = np.full(cap, -1.0, np.float32)
                nv = np.zeros(cap, np.float32)
                iv[:ns] = ksrc_half[m].astype(np.int16)
                dv[:ns] = (kloc[m] % TILE_E).astype(np.float32)
                nv[:ns] = knorm[m]
                idx_h.append(iv)
                doff_h.append(dv)
                norm_h.append(nv)
            idx_h = np.concatenate(idx_h)
            doff_h = np.concatenate(doff_h)
            norm_h = np.concatenate(norm_h)
            cap_h = ncalls_h[h] * call_tiles * TILE_E
            iv = np.zeros(cap_h, np.int16)
            dv = np.full(cap_h, -1.0, np.float32)
            nv = np.zeros(cap_h, np.float32)
            iv[: idx_h.size] = idx_h
            dv[: doff_h.size] = doff_h
            nv[: norm_h.size] = norm_h
            gidx_cols.append(iv)
            doff_cols.append(dv)
            norm_cols.append(nv)

        gidx_flat = np.concatenate(gidx_cols)
        wrapped = gidx_flat.reshape(-1, 16).T.copy()
        gidx = np.tile(wrapped, (8, 1))                        # [128, *]

        dofftab = (
            np.concatenate(doff_cols).reshape(-1, TILE_E).T.astype(np.float32)
        )
        normtab = (
            np.concatenate(norm_cols).reshape(-1, TILE_E).T.astype(np.float32)
        )

        kn = core_of_n == k
        locs = loc_of_n[kn]
        xT = np.zeros((D_IN, npc), np.float32)
        xT[:, locs] = x[kn].T
        dinv_loc = np.zeros(npc, np.float32)
        dinv_loc[locs] = dinv_n[kn]
        maskoff = np.where(dinv_loc > 0, 0.0, NEG_BIG).astype(np.float32)
        maskoff = np.broadcast_to(maskoff, (D, npc)).copy()
        Gt = np.zeros((NB, PART, PART), np.float32)
        li = np.arange(npc)
        pg = li // S
        real = dinv_loc > 0
        Gt[li[real] // PART, li[real] % PART, pg[real]] = 1.0
        kcounts = counts[k * gpc : (k + 1) * gpc].astype(np.float32)
        cntinv = (1.0 / kcounts).reshape(PART, 1).astype(np.float32)

        per_core.append(
            dict(
                xT=xT,
                gidx=gidx,
                dofftab=dofftab,
                normtab=normtab,
                maskoff=maskoff,
                Gt=Gt,
                cntinv=cntinv,
            )
        )

    cfg = dict(
        N=N, E=edge_index.shape[1], G=G, D_IN=D_IN, S=S, npc=npc, NB=NB,
        NT=NT, half=half, tiles_hb=tiles_hb.tolist(), ncalls_h=ncalls_h,
        TT=TT, call_tiles=call_tiles,
        gidx_cols=int(per_core[0]["gidx"].shape[1]),
    )
    return cfg, per_core


def to_bf16(a):
    import ml_dtypes
    return np.asarray(a, dtype=ml_dtypes.bfloat16)


def shared_weights(W0, b0, W1, b1, W2, b2, W3, b3, Wout, bout):
    iota = np.broadcast_to(
        np.arange(TILE_E, dtype=np.float32), (PART, TILE_E)
    ).copy()
    ident = np.eye(PART, dtype=np.float32)
    out = dict(
        iota128=iota, ident=ident,
        w0=np.asarray(W0, np.float32), w1=np.asarray(W1, np.float32),
        w2=np.asarray(W2, np.float32), w3=np.asarray(W3, np.float32),
        b0=np.asarray(b0, np.float32).reshape(D, 1),
        b1=np.asarray(b1, np.float32).reshape(D, 1),
        b2=np.asarray(b2, np.float32).reshape(D, 1),
        b3=np.asarray(b3, np.float32).reshape(D, 1),
        woutA=np.asarray(Wout[:D], np.float32).reshape(D, 1),
        woutB=np.asarray(Wout[D:], np.float32).reshape(D, 1),
        boutrep=np.full((PART, 1), float(np.asarray(bout).ravel()[0]), np.float32),
    )
    return out


# ======================= walrus-compat BIR fixups ======================


_ctr = [0]


def _split_waits(nc):
    for f in nc.m.functions:
        for bb in f.blocks:
            insts = list(bb.instructions)
            out = []
            changed = False
            for ins in insts:
                si = ins.sync_info
                waits = list(si.on_wait) if si is not None and si.on_wait else []
                is_drain = isinstance(ins, mybir.InstDrain)
                keep = 0 if is_drain else 1
                if len(waits) > keep:
                    hoist = waits if is_drain else waits[:-1]
                    for w in hoist:
                        _ctr[0] += 1
                        nop = mybir.InstNoOp(
                            name=f"waitfix_{_ctr[0]}",
                            engine=ins.engine,
                            ins=[],
                            outs=[],
                            sync_info=mybir.SyncInfo(on_wait=[w], on_update=[]),
                            text_hint="waitfix",
                            bass_nofuse=True,
                        )
                        nc.register_instruction(nop, overwrite=True)
                        out.append(nop)
                    kept = [] if is_drain else [waits[-1]]
                    ins.sync_info = mybir.SyncInfo(
                        on_wait=kept, on_update=list(si.on_update or [])
                    )
                    changed = True
                out.append(ins)
            if changed:
                bb.instructions.clear()
                for i in out:
                    bb.instructions.append(i)


def fix_kernel(nc):
    lower_extended_insts(nc)
    _split_waits(nc)
    return nc


# ======================= bass program builder ==========================


def build_gcn(cfg, repeat=1, single_packet=True, gbufs=10, ohbufs=16):
    S = cfg["S"]
    npc = cfg["npc"]
    NB = cfg["NB"]
    NT = cfg["NT"]
    half = cfg["half"]
    D_IN = cfg["D_IN"]
    tiles_hb = cfg["tiles_hb"]
    ncalls_h = cfg["ncalls_h"]
    gidx_cols = cfg["gidx_cols"]
    TT = cfg["TT"]
    CALL_TILES = cfg["call_tiles"]
    CALL_IDX = CALL_TILES * TILE_E
    ncores = NCORES
    f32 = mybir.dt.float32
    bf16 = mybir.dt.bfloat16

    nc = bass.Bass(num_devices=ncores)

    P_xT = nc.declare_dram_parameter("xT", [D_IN, npc], bf16, isOutput=False)
    P_gidx = nc.declare_dram_parameter("gidx", [PART, gidx_cols], mybir.dt.int16, isOutput=False)
    P_doff = nc.declare_dram_parameter("dofftab", [PART, TT], f32, isOutput=False)
    P_norm = nc.declare_dram_parameter("normtab", [PART, TT], f32, isOutput=False)
    P_maskoff = nc.declare_dram_parameter("maskoff", [D, npc], bf16, isOutput=False)
    P_Gt = nc.declare_dram_parameter("Gt", [NB, PART, PART], bf16, isOutput=False)
    P_cntinv = nc.declare_dram_parameter("cntinv", [PART, 1], f32, isOutput=False)
    P_iota = nc.declare_dram_parameter("iota128", [PART, TILE_E], bf16, isOutput=False)
    P_ident = nc.declare_dram_parameter("ident", [PART, PART], bf16, isOutput=False)
    P_w = [nc.declare_dram_parameter(f"w{i}", [D_IN if i == 0 else D, D], bf16, isOutput=False) for i in range(4)]
    P_b = [nc.declare_dram_parameter(f"b{i}", [D, 1], f32, isOutput=False) for i in range(4)]
    P_woutA = nc.declare_dram_parameter("woutA", [D, 1], f32, isOutput=False)
    P_woutB = nc.declare_dram_parameter("woutB", [D, 1], f32, isOutput=False)
    P_boutrep = nc.declare_dram_parameter("boutrep", [PART, 1], f32, isOutput=False)
    P_out = nc.declare_dram_parameter("out", [PART, 1], f32, isOutput=True)

    cc_in = nc.dram_tensor("cc_in", [npc, DP], bf16)
    table = nc.dram_tensor("table", [NT, DP], bf16, addr_space="Shared")

    groups = [list(range(ncores))]

    with tile.TileContext(nc) as tc:
        with ExitStack() as ctx:
            const = ctx.enter_context(tc.tile_pool(name="const", bufs=1))
            big = ctx.enter_context(tc.tile_pool(name="big", bufs=1))
            gpool = ctx.enter_context(tc.tile_pool(name="gbuf", bufs=gbufs))
            ohpool = ctx.enter_context(tc.tile_pool(name="oh", bufs=ohbufs))
            gtp = ctx.enter_context(tc.tile_pool(name="gtp", bufs=2))
            zps = ctx.enter_context(tc.tile_pool(name="zps", bufs=2, space="PSUM"))
            aps = ctx.enter_context(tc.tile_pool(name="aps", bufs=6, space="PSUM"))

            nc.gpsimd.load_library(mlp)
            nidx_reg = nc.gpsimd.to_reg(CALL_IDX)

            def load(pool, shape, dt, src, name):
                t = pool.tile(shape, dt, name=name)
                nc.sync.dma_start(out=t[:], in_=src[:])
                return t

            xT = load(big, [D_IN, npc], bf16, P_xT, "xT")
            gidx = load(const, [PART, gidx_cols], mybir.dt.int16, P_gidx, "gidx")
            dofftab = load(const, [PART, TT], f32, P_doff, "dofftab")
            normtab = load(const, [PART, TT], f32, P_norm, "normtab")
            maskoff = load(big, [D, npc], bf16, P_maskoff, "maskoff")
            cntinv = load(const, [PART, 1], f32, P_cntinv, "cntinv")
            iota128 = load(const, [PART, TILE_E], bf16, P_iota, "iota128")
            ident = load(const, [PART, PART], bf16, P_ident, "ident")
            w_sb = [load(const, [D_IN if i == 0 else D, D], bf16, P_w[i], f"w{i}") for i in range(4)]
            b_sb = [load(const, [D, 1], f32, P_b[i], f"b{i}") for i in range(4)]
            woutA = load(const, [D, 1], f32, P_woutA, "woutA")
            woutB = load(const, [D, 1], f32, P_woutB, "woutB")
            boutrep = load(const, [PART, 1], f32, P_boutrep, "boutrep")

            h_sb = big.tile([D, npc], bf16, name="h_sb")
            zloc = big.tile([PART, S, DP], bf16, name="zloc")
            nc.vector.memset(zloc[:], 0.0)

            last_gathers = []

            for L in [l % 4 for l in range(4 * repeat)]:
                # ---- z' = h @ W_L (no scaling; norm folded into one-hots) ----
                src_t = xT if L == 0 else h_sb
                src_v = src_t[:].rearrange("f (p c) -> f c p", c=S)
                for c in range(S):
                    zp = zps.tile([PART, D], f32, name="zp")
                    nc.tensor.matmul(
                        out=zp[:], lhsT=src_v[:, c, :], rhs=w_sb[L][:],
                        start=True, stop=True,
                    )
                    nc.vector.tensor_copy(out=zloc[:, c, 0:D], in_=zp[:])
                dma_out = nc.sync.dma_start(
                    out=cc_in[:].rearrange("(p c) f -> p (c f)", p=PART),
                    in_=zloc[:],
                )
                ag = nc.gpsimd.collective_compute(
                    "AllGather",
                    mybir.AluOpType.bypass,
                    replica_groups=groups,
                    ins=[cc_in[:]],
                    outs=[table[:]],
                )
                add_dep_helper(ag.ins, dma_out.ins, reason="cc_in ready")
                for g in last_gathers:
                    add_dep_helper(ag.ins, g.ins, reason="table WAR")
                last_gathers = []

                # ---- gather + aggregate; both halves into one PSUM/block ----
                tbl_half = [table[0:half, :], table[half:NT, :]]
                col_base = [0, ncalls_h[0] * (CALL_IDX // 16)]
                tile_base = [0, ncalls_h[0] * CALL_TILES]
                gbufs = [{}, {}]
                tglob = [0, 0]
                for b in range(NB):
                    ap_ps = aps.tile([D, TILE_E], f32, name="agg")
                    ntile0, ntile1 = tiles_hb[0][b], tiles_hb[1][b]
                    for h in (0, 1):
                        ntile = tiles_hb[h][b]
                        for ti in range(ntile):
                            tg = tglob[h]
                            call = tg // CALL_TILES
                            slot = tg % CALL_TILES
                            if call not in gbufs[h]:
                                gb = gpool.tile(
                                    [PART, CALL_TILES, DP], bf16, name="gb"
                                )
                                cb = col_base[h] + call * (CALL_IDX // 16)
                                gi = nc.gpsimd.dma_gather(
                                    gb[:], tbl_half[h],
                                    gidx[:, cb : cb + CALL_IDX // 16],
                                    CALL_IDX, nidx_reg, DP,
                                    single_packet=single_packet,
                                )
                                add_dep_helper(gi.ins, ag.ins, reason="table ready")
                                last_gathers.append(gi)
                                gbufs[h][call] = gb
                                if len(gbufs[h]) > 20:
                                    del gbufs[h][min(gbufs[h])]
                            gb = gbufs[h][call]
                            tcol = tile_base[h] + tg
                            oh = ohpool.tile([PART, TILE_E], bf16, name="oh")
                            nc.vector.tensor_scalar(
                                out=oh[:], in0=iota128[:],
                                scalar1=dofftab[:, tcol : tcol + 1],
                                scalar2=normtab[:, tcol : tcol + 1],
                                op0=mybir.AluOpType.is_equal,
                                op1=mybir.AluOpType.mult,
                            )
                            nc.tensor.matmul(
                                out=ap_ps[:], lhsT=gb[:, slot, 0:D], rhs=oh[:],
                                start=(h == 0 and ti == 0),
                                stop=(h == 1 and ti == ntile1 - 1),
                            )
                            tglob[h] += 1
                    cs = slice(b * TILE_E, (b + 1) * TILE_E)
                    nc.scalar.activation(
                        out=h_sb[:, cs], in_=ap_ps[:],
                        func=mybir.ActivationFunctionType.Tanh,
                        bias=b_sb[L][:, 0:1],
                    )

            # ---- readout ----
            macc = big.tile([D, npc], bf16, name="macc")
            nc.vector.tensor_tensor(
                out=macc[:], in0=h_sb[:], in1=maskoff[:], op=mybir.AluOpType.add
            )
            gmax = const.tile([D, PART], f32, name="gmax")
            for g in range(PART):
                nc.vector.tensor_reduce(
                    out=gmax[:, g : g + 1],
                    in_=macc[:, g * S : (g + 1) * S],
                    axis=mybir.AxisListType.X,
                    op=mybir.AluOpType.max,
                )
            hT = big.tile([PART, NB, D], bf16, name="hT")
            for b in range(NB):
                tp = zps.tile([PART, D], bf16, name="tp", tag="zp")
                nc.tensor.transpose(
                    out=tp[:], in_=h_sb[:, b * TILE_E : (b + 1) * TILE_E],
                    identity=ident[0:D, 0:D],
                )
                nc.scalar.activation(
                    out=hT[:, b, :], in_=tp[:],
                    func=mybir.ActivationFunctionType.Copy,
                )
            gsum_ps = aps.tile([D, PART], f32, name="gsum_ps", tag="agg")
            for b in range(NB):
                gt = gtp.tile([PART, PART], bf16, name="gt")
                nc.sync.dma_start(out=gt[:], in_=P_Gt[b, :, :])
                nc.tensor.matmul(
                    out=gsum_ps[:], lhsT=hT[:, b, :], rhs=gt[:],
                    start=(b == 0), stop=(b == NB - 1),
                )
            gsumT = const.tile([D, PART], f32, name="gsumT")
            nc.vector.tensor_copy(out=gsumT[:], in_=gsum_ps[:])

            f1 = aps.tile([PART, 1], f32, name="f1", tag="agg")
            nc.tensor.matmul(out=f1[:], lhsT=gmax[:], rhs=woutA[:], start=True, stop=True)
            f2 = aps.tile([PART, 1], f32, name="f2", tag="agg")
            nc.tensor.matmul(out=f2[:], lhsT=gsumT[:], rhs=woutB[:], start=True, stop=True)
            o1 = const.tile([PART, 1], f32, name="o1")
            nc.vector.tensor_tensor(out=o1[:], in0=f2[:], in1=cntinv[:], op=mybir.AluOpType.mult)
            o2 = const.tile([PART, 1], f32, name="o2")
            nc.vector.tensor_tensor(out=o2[:], in0=o1[:], in1=f1[:], op=mybir.AluOpType.add)
            o3 = const.tile([PART, 1], f32, name="o3")
            nc.vector.tensor_tensor(out=o3[:], in0=o2[:], in1=boutrep[:], op=mybir.AluOpType.add)
            nc.sync.dma_start(out=P_out[:], in_=o3[:])

    return nc


def make_in_maps(per_core, shared):
    """Cast host arrays to the dtypes the program declares."""
    import ml_dtypes
    bf = ml_dtypes.bfloat16
    maps = []
    for pc in per_core:
        m = dict(
            xT=pc["xT"].astype(bf),
            gidx=pc["gidx"],
            dofftab=pc["dofftab"],
            normtab=pc["normtab"],
            maskoff=pc["maskoff"].astype(bf),
            Gt=pc["Gt"].astype(bf),
            cntinv=pc["cntinv"],
            iota128=shared["iota128"].astype(bf),
            ident=shared["ident"].astype(bf),
            w0=shared["w0"].astype(bf), w1=shared["w1"].astype(bf),
            w2=shared["w2"].astype(bf), w3=shared["w3"].astype(bf),
            b0=shared["b0"], b1=shared["b1"],
            b2=shared["b2"], b3=shared["b3"],
            woutA=shared["woutA"], woutB=shared["woutB"],
            boutrep=shared["boutrep"],
        )
        maps.append(m)
    return maps


_CACHE = {}


def kernel(x, edge_index, batch, W0, b0, W1, b1, W2, b2, W3, b3, Wout, bout):
    from concourse.bass_utils import run_bass_kernel_spmd

    x = np.asarray(x, np.float32)
    edge_index = np.asarray(edge_index)
    batch = np.asarray(batch)

    cfg, per_core = preprocess(x, edge_index, batch)
    shared = shared_weights(W0, b0, W1, b1, W2, b2, W3, b3, Wout, bout)

    key = cfg["gidx_cols"], cfg["TT"], tuple(map(tuple, cfg["tiles_hb"]))
    nc = _CACHE.get(key)
    if nc is None:
        nc = build_gcn(cfg)
        fix_kernel(nc)
        _CACHE[key] = nc

    in_maps = make_in_maps(per_core, shared)
    res = run_bass_kernel_spmd(nc, in_maps, list(range(8)))
    out = np.concatenate([res.results[k]["out"].reshape(-1) for k in range(8)])
    return out[:, None].astype(np.float32)



# revision 14
# speedup vs baseline: 3.5977x; 3.5977x over previous
"""Optimized Trainium2 Bass kernel for nn_GCN_11269994185058 (v4).

Key design (evolved from v2/v3 via real-HW ablation):
- Graphs partitioned across 8 cores (128 graphs each); nodes laid out
  graph-major with per-graph stride S. Per layer: local z = h @ W, AllGather
  of the bf16 feature table (rows padded to 128 cols = 256B gather
  granularity), then per-dest-block aggregation as one-hot matmuls
  accumulating in PSUM, tanh+bias via ScalarE.
- dma_gather calls spread over 4 SWDGE queues (18x descriptor throughput vs
  one queue: per-queue ring fits one 1024-desc call, so a single queue
  serializes desc-gen against transfer drain).
- Gather pad slots point at scattered rows, not row 0 (16 engines hitting
  one 256B address serialize HBM).
- One-hot matrices are CONSTANT per tile and identical across layers: built
  on the host, streamed from DRAM in 64-tile arenas (double-buffered) on the
  HWDGE path. This removes the per-tile DVE->PE dependency that cost ~1ms
  per block when one-hots were built on-device.
- zloc is block-major: z for dest-block b is computed from h_sb columns
  b*128..(b+1)*128 right after that block's tanh. Reference self-loops are
  aggregated as LOCAL diagonal tiles (lhsT = zloc slice, no gather).
"""

from contextlib import ExitStack

import numpy as np

import concourse.bass as bass
import concourse.mybir as mybir
from concourse import tile
from concourse.library_config import mlp
from concourse.library_overlay import lower_extended_insts
from concourse.tile import add_dep_helper

NCORES = 8
D = 64
DP = 128             # padded table row width (bf16) -> 256B
PART = 128
TILE_E = 128
ARENA = 64           # one-hot tiles per streamed arena
NEG_BIG = -1.0e30
import os
SHUFFLE_IN_GROUP = bool(int(os.environ.get('SHUF', '0')))


# ======================= host-side preprocessing =======================


def preprocess(x, edge_index, batch, call_tiles=8):
    N, D_IN = x.shape
    G = int(batch.max()) + 1
    gpc = G // NCORES
    assert gpc == PART

    counts = np.bincount(batch, minlength=G)
    S = int(counts.max())
    npc = PART * S
    NB = npc // PART
    NT = NCORES * npc
    nrows = NT // 2          # pair-packed table rows (2 nodes per 256B row)
    assert nrows <= 32768

    gstart = np.zeros(G + 1, np.int64)
    np.cumsum(counts, out=gstart[1:])
    g_of_n = batch.astype(np.int64)
    j_of_n = np.arange(N) - gstart[g_of_n]
    core_of_n = g_of_n // gpc
    loc_of_n = (g_of_n % gpc) * S + j_of_n
    pgid_of_n = core_of_n * npc + loc_of_n

    deg = np.bincount(edge_index[1], minlength=N).astype(np.float64) + 1.0
    dinv_n = (1.0 / np.sqrt(deg)).astype(np.float32)

    # Random graph edges go through the gather path; the reference's
    # appended self-loops are handled locally as diagonal tiles.
    src = edge_index[0]
    dst = edge_index[1]
    e_norm = (dinv_n[src] * dinv_n[dst]).astype(np.float32)
    e_src_pgid = pgid_of_n[src]
    e_dst_core = core_of_n[dst]
    e_dst_loc = loc_of_n[dst]
    e_half = e_src_pgid % 2          # parity group: which 128B half of the row
    e_block = e_dst_loc // TILE_E

    cnt = np.zeros((NCORES, 2, NB), np.int64)
    np.add.at(cnt, (e_dst_core, e_half, e_block), 1)
    tiles_hb = np.maximum(1, -(-cnt.max(axis=0) // TILE_E))    # [2, NB]

    tiles_per_half = tiles_hb.sum(axis=1)
    ncalls_h = [int(-(-t // call_tiles)) for t in tiles_per_half]

    # one-hot consumption stream: per block: self tile, then h0, then h1
    n_stream = int(NB + tiles_hb.sum())
    NA = -(-n_stream // ARENA)

    per_core = []
    for k in range(NCORES):
        pad_rng = np.random.default_rng(12345 + k)
        sel = e_dst_core == k
        ksrc = e_src_pgid[sel]
        kloc = e_dst_loc[sel]
        khalf = e_half[sel]
        kblk = e_block[sel]
        knorm = e_norm[sel]
        ksrc_half = ksrc // 2        # pair-row index
        if SHUFFLE_IN_GROUP:
            jitter = pad_rng.random(ksrc.size)
            order = np.lexsort((jitter, kblk, khalf))
        else:
            order = np.lexsort((ksrc, kblk, khalf))
        ksrc_half = ksrc_half[order]
        kloc = kloc[order]
        khalf = khalf[order]
        kblk = kblk[order]
        knorm = knorm[order]

        # per-(h,b) padded slot arrays
        gidx_cols = []
        oh_hb = {}
        for h in (0, 1):
            idx_h = []
            for b in range(NB):
                m = (khalf == h) & (kblk == b)
                ns = int(m.sum())
                cap = int(tiles_hb[h, b]) * TILE_E
                assert ns <= cap
                iv = pad_rng.integers(0, nrows, cap).astype(np.int16)
                dv = np.full(cap, -1, np.int64)
                nv = np.zeros(cap, np.float32)
                iv[:ns] = ksrc_half[m].astype(np.int16)
                dv[:ns] = kloc[m] % TILE_E
                nv[:ns] = knorm[m]
                idx_h.append(iv)
                oh_list = []
                for t in range(int(tiles_hb[h, b])):
                    dvt = dv[t * TILE_E:(t + 1) * TILE_E]
                    nvt = nv[t * TILE_E:(t + 1) * TILE_E]
                    oh = np.zeros((TILE_E, TILE_E), np.float32)
                    rows = np.nonzero(dvt >= 0)[0]
                    oh[rows, dvt[rows]] = nvt[rows]
                    oh_list.append(oh)
                oh_hb[(h, b)] = oh_list
            idx_h = np.concatenate(idx_h)
            cap_h = ncalls_h[h] * call_tiles * TILE_E
            iv = pad_rng.integers(0, nrows, cap_h).astype(np.int16)
            iv[: idx_h.size] = idx_h
            gidx_cols.append(iv)

        gidx_flat = np.concatenate(gidx_cols)
        wrapped = gidx_flat.reshape(-1, 16).T.copy()
        gidx = np.tile(wrapped, (8, 1))                        # [128, *]

        # nodes on this core
        kn = core_of_n == k
        locs = loc_of_n[kn]
        xT = np.zeros((D_IN, npc), np.float32)
        xT[:, locs] = x[kn].T
        dinv_loc = np.zeros(npc, np.float32)
        dinv_loc[locs] = dinv_n[kn]

        # one-hot stream (consumption order): per block: self diag, h0, h1
        ohtab = np.zeros((NA * ARENA, TILE_E, TILE_E), np.float32)
        si = 0
        for b in range(NB):
            diag = np.zeros((TILE_E, TILE_E), np.float32)
            dv = dinv_loc[b * TILE_E:(b + 1) * TILE_E]
            np.fill_diagonal(diag, dv * dv)
            ohtab[si] = diag
            si += 1
            for h in (0, 1):
                for oh in oh_hb[(h, b)]:
                    ohtab[si] = oh
                    si += 1
        assert si == n_stream
        ohtab = ohtab.reshape(NA, ARENA, TILE_E, TILE_E)
        ohtab = np.ascontiguousarray(ohtab.transpose(0, 2, 1, 3))  # [NA,128,ARENA,128]

        maskoff = np.where(dinv_loc > 0, 0.0, NEG_BIG).astype(np.float32)
        maskoff = np.broadcast_to(maskoff, (D, npc)).copy()
        Gt = np.zeros((NB, PART, PART), np.float32)
        li = np.arange(npc)
        pg = li // S
        real = dinv_loc > 0
        Gt[li[real] // PART, li[real] % PART, pg[real]] = 1.0
        kcounts = counts[k * gpc : (k + 1) * gpc].astype(np.float32)
        cntinv = (1.0 / kcounts).reshape(PART, 1).astype(np.float32)

        per_core.append(
            dict(
                xT=xT,
                gidx=gidx,
                ohtab=ohtab,
                maskoff=maskoff,
                Gt=Gt,
                cntinv=cntinv,
            )
        )

    cfg = dict(
        N=N, E=edge_index.shape[1], G=G, D_IN=D_IN, S=S, npc=npc, NB=NB,
        NT=NT, nrows=nrows, tiles_hb=tiles_hb.tolist(), ncalls_h=ncalls_h,
        NA=NA, n_stream=n_stream, call_tiles=call_tiles,
        gidx_cols=int(per_core[0]["gidx"].shape[1]),
    )
    return cfg, per_core


def to_bf16(a):
    import ml_dtypes
    return np.asarray(a, dtype=ml_dtypes.bfloat16)


def shared_weights(W0, b0, W1, b1, W2, b2, W3, b3, Wout, bout):
    ident = np.eye(PART, dtype=np.float32)
    out = dict(
        ident=ident,
        w0=np.asarray(W0, np.float32), w1=np.asarray(W1, np.float32),
        w2=np.asarray(W2, np.float32), w3=np.asarray(W3, np.float32),
        b0=np.asarray(b0, np.float32).reshape(D, 1),
        b1=np.asarray(b1, np.float32).reshape(D, 1),
        b2=np.asarray(b2, np.float32).reshape(D, 1),
        b3=np.asarray(b3, np.float32).reshape(D, 1),
        woutA=np.asarray(Wout[:D], np.float32).reshape(D, 1),
        woutB=np.asarray(Wout[D:], np.float32).reshape(D, 1),
        boutrep=np.full((PART, 1), float(np.asarray(bout).ravel()[0]), np.float32),
    )
    return out


# ======================= walrus-compat BIR fixups ======================


_ctr = [0]


def _split_waits(nc):
    for f in nc.m.functions:
        for bb in f.blocks:
            insts = list(bb.instructions)
            out = []
            changed = False
            for ins in insts:
                si = ins.sync_info
                waits = list(si.on_wait) if si is not None and si.on_wait else []
                is_drain = isinstance(ins, mybir.InstDrain)
                keep = 0 if is_drain else 1
                if len(waits) > keep:
                    hoist = waits if is_drain else waits[:-1]
                    for w in hoist:
                        _ctr[0] += 1
                        nop = mybir.InstNoOp(
                            name=f"waitfix_{_ctr[0]}",
                            engine=ins.engine,
                            ins=[],
                            outs=[],
                            sync_info=mybir.SyncInfo(on_wait=[w], on_update=[]),
                            text_hint="waitfix",
                            bass_nofuse=True,
                        )
                        nc.register_instruction(nop, overwrite=True)
                        out.append(nop)
                    kept = [] if is_drain else [waits[-1]]
                    ins.sync_info = mybir.SyncInfo(
                        on_wait=kept, on_update=list(si.on_update or [])
                    )
                    changed = True
                out.append(ins)
            if changed:
                bb.instructions.clear()
                for i in out:
                    bb.instructions.append(i)


def fix_kernel(nc):
    lower_extended_insts(nc)
    _split_waits(nc)
    return nc


# ======================= bass program builder ==========================


def build_gcn(cfg, repeat=1, single_packet=True, gbufs=12, arenabufs=3, nq=4, zlag=3, zpsbufs=2, interleave_z=1):
    S = cfg["S"]
    npc = cfg["npc"]
    NB = cfg["NB"]
    NT = cfg["NT"]
    nrows = cfg["nrows"]
    D_IN = cfg["D_IN"]
    tiles_hb = cfg["tiles_hb"]
    ncalls_h = cfg["ncalls_h"]
    gidx_cols = cfg["gidx_cols"]
    NA = cfg["NA"]
    CALL_TILES = cfg["call_tiles"]
    CALL_IDX = CALL_TILES * TILE_E
    ncores = NCORES
    f32 = mybir.dt.float32
    bf16 = mybir.dt.bfloat16

    nc = bass.Bass(num_devices=ncores, num_swdge_queues=nq)

    P_xT = nc.declare_dram_parameter("xT", [D_IN, npc], bf16, isOutput=False)
    P_gidx = nc.declare_dram_parameter("gidx", [PART, gidx_cols], mybir.dt.int16, isOutput=False)
    P_oh = nc.declare_dram_parameter("ohtab", [NA, PART, ARENA * TILE_E], bf16, isOutput=False)
    P_maskoff = nc.declare_dram_parameter("maskoff", [D, npc], bf16, isOutput=False)
    P_Gt = nc.declare_dram_parameter("Gt", [NB, PART, PART], bf16, isOutput=False)
    P_cntinv = nc.declare_dram_parameter("cntinv", [PART, 1], f32, isOutput=False)
    P_ident = nc.declare_dram_parameter("ident", [PART, PART], bf16, isOutput=False)
    P_w = [nc.declare_dram_parameter(f"w{i}", [D_IN if i == 0 else D, D], bf16, isOutput=False) for i in range(4)]
    P_b = [nc.declare_dram_parameter(f"b{i}", [D, 1], f32, isOutput=False) for i in range(4)]
    P_woutA = nc.declare_dram_parameter("woutA", [D, 1], f32, isOutput=False)
    P_woutB = nc.declare_dram_parameter("woutB", [D, 1], f32, isOutput=False)
    P_boutrep = nc.declare_dram_parameter("boutrep", [PART, 1], f32, isOutput=False)
    P_out = nc.declare_dram_parameter("out", [PART, 1], f32, isOutput=True)

    cc_in = nc.dram_tensor("cc_in", [npc // 2, DP], bf16)
    table = nc.dram_tensor("table", [nrows, DP], bf16, addr_space="Shared")

    groups = [list(range(ncores))]

    with tile.TileContext(nc) as tc:
        with ExitStack() as ctx:
            const = ctx.enter_context(tc.tile_pool(name="const", bufs=1))
            big = ctx.enter_context(tc.tile_pool(name="big", bufs=1))
            gpool = ctx.enter_context(tc.tile_pool(name="gbuf", bufs=gbufs))
            ohpool = ctx.enter_context(tc.tile_pool(name="oha", bufs=arenabufs))
            gtp = ctx.enter_context(tc.tile_pool(name="gtp", bufs=2))
            zps = ctx.enter_context(tc.tile_pool(name="zps", bufs=zpsbufs, space="PSUM"))
            aps = ctx.enter_context(tc.tile_pool(name="aps", bufs=6, space="PSUM"))

            nc.gpsimd.load_library(mlp)
            nidx_reg = nc.gpsimd.to_reg(CALL_IDX)
            qctr = [0]

            def load(pool, shape, dt, src, name):
                t = pool.tile(shape, dt, name=name)
                nc.sync.dma_start(out=t[:], in_=src[:])
                return t

            xT = load(big, [D_IN, npc], bf16, P_xT, "xT")
            gidx = load(const, [PART, gidx_cols], mybir.dt.int16, P_gidx, "gidx")
            maskoff = load(big, [D, npc], bf16, P_maskoff, "maskoff")
            cntinv = load(const, [PART, 1], f32, P_cntinv, "cntinv")
            ident = load(const, [PART, PART], bf16, P_ident, "ident")
            w_sb = [load(const, [D_IN if i == 0 else D, D], bf16, P_w[i], f"w{i}") for i in range(4)]
            b_sb = [load(const, [D, 1], f32, P_b[i], f"b{i}") for i in range(4)]
            woutA = load(const, [D, 1], f32, P_woutA, "woutA")
            woutB = load(const, [D, 1], f32, P_woutB, "woutB")
            boutrep = load(const, [PART, 1], f32, P_boutrep, "boutrep")

            h_sb = big.tile([D, npc], bf16, name="h_sb")
            zloc = big.tile([PART, NB, D], bf16, name="zloc")
            nc.vector.memset(zloc[:], 0.0)

            last_gathers = []
            ZLAG = zlag
            nlayers = 4 * repeat

            def emit_z(src_t, b, wl):
                zp = zps.tile([PART, D], f32, name="zp")
                nc.tensor.matmul(
                    out=zp[:], lhsT=src_t[:, b * PART:(b + 1) * PART],
                    rhs=w_sb[wl][:], start=True, stop=True,
                )
                nc.vector.tensor_copy(out=zloc[:, b, :], in_=zp[:])

            def emit_cc():
                # pair-fold: node loc=c*128+p lands at pair-row (c*64+p//2),
                # column half (p%2)*64; per-partition offset is linear: p*64
                dma_out = nc.sync.dma_start(
                    out=cc_in[:].rearrange("(c q) (e f) -> (q e) c f", q=64, e=2),
                    in_=zloc[:],
                )
                ag = nc.gpsimd.collective_compute(
                    "AllGather",
                    mybir.AluOpType.bypass,
                    replica_groups=groups,
                    ins=[cc_in[:]],
                    outs=[table[:]],
                )
                add_dep_helper(ag.ins, dma_out.ins, reason="cc_in ready")
                for g in last_gathers:
                    add_dep_helper(ag.ins, g.ins, reason="table WAR")
                last_gathers.clear()
                return ag

            # layer-0 z from xT, then first AllGather
            for b in range(NB):
                emit_z(xT, b, 0)
            ag = emit_cc()

            for l in range(nlayers):
                L = l % 4
                wl_next = (l + 1) % 4
                is_last = l == nlayers - 1

                # ---- gather + aggregate ----
                col_base = [0, ncalls_h[0] * (CALL_IDX // 16)]
                gbufs_d = [{}, {}]
                tglob = [0, 0]
                sctr = [0]          # one-hot stream position
                arena = [None]

                def next_oh():
                    s = sctr[0]
                    a, j = divmod(s, ARENA)
                    if j == 0:
                        at = ohpool.tile([PART, ARENA, TILE_E], bf16, name="oha")
                        nc.sync.dma_start(
                            out=at[:],
                            in_=P_oh[a, :, :].rearrange("p (t e) -> p t e", t=ARENA),
                        )
                        arena[0] = at
                    sctr[0] += 1
                    return arena[0][:, j, :]

                for b in range(NB):
                    ap_ps = aps.tile([D, TILE_E], f32, name="agg")
                    ntile1 = tiles_hb[1][b]
                    # self-loop diagonal tile (local z, no gather)
                    oh = next_oh()
                    nc.tensor.matmul(
                        out=ap_ps[:], lhsT=zloc[:, b, :], rhs=oh,
                        start=True, stop=False,
                    )
                    for h in (0, 1):
                        ntile = tiles_hb[h][b]
                        for ti in range(ntile):
                            tg = tglob[h]
                            call = tg // CALL_TILES
                            slot = tg % CALL_TILES
                            if call not in gbufs_d[h]:
                                gb = gpool.tile(
                                    [PART, CALL_TILES, DP], bf16, name="gb"
                                )
                                cb = col_base[h] + call * (CALL_IDX // 16)
                                gi = nc.gpsimd.dma_gather(
                                    gb[:], table[:],
                                    gidx[:, cb : cb + CALL_IDX // 16],
                                    CALL_IDX, nidx_reg, DP,
                                    single_packet=single_packet,
                                    queue_num=qctr[0] % nq,
                                )
                                qctr[0] += 1
                                add_dep_helper(gi.ins, ag.ins, reason="table ready")
                                last_gathers.append(gi)
                                gbufs_d[h][call] = gb
                                if len(gbufs_d[h]) > 20:
                                    del gbufs_d[h][min(gbufs_d[h])]
                            gb = gbufs_d[h][call]
                            oh = next_oh()
                            nc.tensor.matmul(
                                out=ap_ps[:],
                                lhsT=gb[:, slot, h * D:(h + 1) * D], rhs=oh,
                                start=False,
                                stop=(h == 1 and ti == ntile1 - 1),
                            )
                            tglob[h] += 1
                    cs = slice(b * TILE_E, (b + 1) * TILE_E)
                    nc.scalar.activation(
                        out=h_sb[:, cs], in_=ap_ps[:],
                        func=mybir.ActivationFunctionType.Tanh,
                        bias=b_sb[L][:, 0:1],
                    )
                    # interleave next layer's z (lagged so PE never waits on
                    # a just-issued tanh)
                    if interleave_z and not is_last and b >= ZLAG:
                        # repeat>1 timing builds cycle back to w0 (9xD): feed
                        # xT so dims match (numerics only matter at repeat=1)
                        emit_z(xT if wl_next == 0 else h_sb, b - ZLAG, wl_next)
                if not is_last:
                    lo = NB - ZLAG if interleave_z else 0
                    for b in range(lo, NB):
                        emit_z(xT if wl_next == 0 else h_sb, b, wl_next)
                    ag = emit_cc()

            # ---- readout ----
            macc = big.tile([D, npc], bf16, name="macc")
            nc.vector.tensor_tensor(
                out=macc[:], in0=h_sb[:], in1=maskoff[:], op=mybir.AluOpType.add
            )
            gmax = const.tile([D, PART], f32, name="gmax")
            for g in range(PART):
                nc.vector.tensor_reduce(
                    out=gmax[:, g : g + 1],
                    in_=macc[:, g * S : (g + 1) * S],
                    axis=mybir.AxisListType.X,
                    op=mybir.AluOpType.max,
                )
            hT = big.tile([PART, NB, D], bf16, name="hT")
            for b in range(NB):
                tp = zps.tile([PART, D], bf16, name="tp", tag="zp")
                nc.tensor.transpose(
                    out=tp[:], in_=h_sb[:, b * TILE_E : (b + 1) * TILE_E],
                    identity=ident[0:D, 0:D],
                )
                nc.scalar.activation(
                    out=hT[:, b, :], in_=tp[:],
                    func=mybir.ActivationFunctionType.Copy,
                )
            gsum_ps = aps.tile([D, PART], f32, name="gsum_ps", tag="agg")
            for b in range(NB):
                gt = gtp.tile([PART, PART], bf16, name="gt")
                nc.sync.dma_start(out=gt[:], in_=P_Gt[b, :, :])
                nc.tensor.matmul(
                    out=gsum_ps[:], lhsT=hT[:, b, :], rhs=gt[:],
                    start=(b == 0), stop=(b == NB - 1),
                )
            gsumT = const.tile([D, PART], f32, name="gsumT")
            nc.vector.tensor_copy(out=gsumT[:], in_=gsum_ps[:])

            f1 = aps.tile([PART, 1], f32, name="f1", tag="agg")
            nc.tensor.matmul(out=f1[:], lhsT=gmax[:], rhs=woutA[:], start=True, stop=True)
            f2 = aps.tile([PART, 1], f32, name="f2", tag="agg")
            nc.tensor.matmul(out=f2[:], lhsT=gsumT[:], rhs=woutB[:], start=True, stop=True)
            o1 = const.tile([PART, 1], f32, name="o1")
            nc.vector.tensor_tensor(out=o1[:], in0=f2[:], in1=cntinv[:], op=mybir.AluOpType.mult)
            o2 = const.tile([PART, 1], f32, name="o2")
            nc.vector.tensor_tensor(out=o2[:], in0=o1[:], in1=f1[:], op=mybir.AluOpType.add)
            o3 = const.tile([PART, 1], f32, name="o3")
            nc.vector.tensor_tensor(out=o3[:], in0=o2[:], in1=boutrep[:], op=mybir.AluOpType.add)
            nc.sync.dma_start(out=P_out[:], in_=o3[:])

    return nc


def make_in_maps(per_core, shared):
    """Cast host arrays to the dtypes the program declares."""
    import ml_dtypes
    bf = ml_dtypes.bfloat16
    maps = []
    for pc in per_core:
        na, p_, cols = pc["ohtab"].shape[0], pc["ohtab"].shape[1], None
        m = dict(
            xT=pc["xT"].astype(bf),
            gidx=pc["gidx"],
            ohtab=pc["ohtab"].reshape(pc["ohtab"].shape[0], PART, -1).astype(bf),
            maskoff=pc["maskoff"].astype(bf),
            Gt=pc["Gt"].astype(bf),
            cntinv=pc["cntinv"],
            ident=shared["ident"].astype(bf),
            w0=shared["w0"].astype(bf), w1=shared["w1"].astype(bf),
            w2=shared["w2"].astype(bf), w3=shared["w3"].astype(bf),
            b0=shared["b0"], b1=shared["b1"],
            b2=shared["b2"], b3=shared["b3"],
            woutA=shared["woutA"], woutB=shared["woutB"],
            boutrep=shared["boutrep"],
        )
        maps.append(m)
    return maps


_CACHE = {}


def kernel(x, edge_index, batch, W0, b0, W1, b1, W2, b2, W3, b3, Wout, bout):
    from concourse.bass_utils import run_bass_kernel_spmd

    x = np.asarray(x, np.float32)
    edge_index = np.asarray(edge_index)
    batch = np.asarray(batch)

    cfg, per_core = preprocess(x, edge_index, batch)
    shared = shared_weights(W0, b0, W1, b1, W2, b2, W3, b3, Wout, bout)

    key = cfg["gidx_cols"], cfg["NA"], tuple(map(tuple, cfg["tiles_hb"])), 5
    nc = _CACHE.get(key)
    if nc is None:
        nc = build_gcn(cfg)
        fix_kernel(nc)
        _CACHE[key] = nc

    in_maps = make_in_maps(per_core, shared)
    res = run_bass_kernel_spmd(nc, in_maps, list(range(8)))
    out = np.concatenate([res.results[k]["out"].reshape(-1) for k in range(8)])
    return out[:, None].astype(np.float32)
